# revision 4
# baseline (speedup 1.0000x reference)
"""Trainium2 Bass kernel for nn_DistMaps: per-image Gaussian click maps.

out[b, c, i, j] = max over valid points p of channel c of
    exp(-((i - px_p)^2 + (j - py_p)^2) / r_p^2),  init 0.

Shapes (hardcoded): x [8, 3, 512, 512] (UNUSED by the math - only its batch
dim matters), coords [8, 48, 4] (px, py, order, radius); points 0..23 ->
channel 0, 24..47 -> channel 1. Output [8, 2, 512, 512] float32.

Strategy: data-parallel across 8 NeuronCores (1 batch image per core).
The Gaussian is separable: exp(-d2/r^2) = exp(-(i-px)^2/r^2) *
exp(-(j-py)^2/r^2), so per point we build row/col 1-D factor tables
A[p, i], B[p, j] (invalid points forced to 0 via a -1e30 exp scale), then
materialize each outer product A[p,:]^T B[p,:] with a K=1 TensorEngine
matmul into PSUM and max-accumulate on the VectorEngine.
"""

import contextlib
import ctypes
import sys
import types

import numpy as np

import concourse.bass as bass
import concourse.mybir as mybir
from concourse import bass_utils
from concourse import tile
from concourse.bass_utils import run_bass_kernel_spmd
from concourse.tile import ScopedClock, TileContext

B, C, H, W, P2 = 8, 3, 512, 512, 48
HALF = P2 // 2  # 24 points per channel
N_CORES = 8
F32 = mybir.dt.float32
F32R = mybir.dt.float32r
I32 = mybir.dt.int32
AF = mybir.ActivationFunctionType
OP = mybir.AluOpType


# --------------------------------------------------------------------------
# Environment patches for this container (axon-tunneled TRN2):
#  1. This walrus build rejects >1 sem-wait on CTRL-class instructions; the
#     stock Tile tail drain carries one wait per outstanding semaphore.
#     Split them one-per-NOP ahead of a bare drain.
#  2. NTFF profiling hook: antenv.axon_hooks is absent in this image; provide
#     it via ctypes against libaxon_pjrt.so so trace=True works.
#  3. Artifact upload has no bucket credentials here; make it a no-op.
# --------------------------------------------------------------------------
def _patched_drain_and_barrier(self, tick_clock, wait_clock):
    nc = self.nc
    probe = nc.sync.nop(nofuse=True, hint="tail_wait_probe")
    wait_clock.add_sem_waits(probe.ins, ScopedClock({None: tick_clock.global_clock}))
    si = probe.ins.sync_info
    waits = list(si.on_wait or []) if si is not None else []
    if len(waits) > 1:
        si.on_wait = waits[:1]
        for w in waits[1:]:
            n = nc.sync.nop(nofuse=True, hint="tail_wait_nop")
            n.ins.sync_info = mybir.SyncInfo(on_wait=[w], on_update=[])
    nc.sync.drain()
    nc.all_engine_barrier()
    assert self.sems is not None
    popped = nc._tile_sem_poison_stack.pop()
    assert popped is self._sem_poison
    nc.clear_and_free_semaphores(list(self.sems.allocated().values()))
    nc.all_engine_barrier()


TileContext._drain_and_barrier = _patched_drain_and_barrier


def _make_ntff_hook(so_path="/opt/axon/libaxon_pjrt.so"):
    try:
        lib = ctypes.CDLL(so_path)
    except OSError:
        return None
    if not hasattr(lib, "axon_start_nrt_profile"):
        return None
    lib.axon_start_nrt_profile.argtypes = [
        ctypes.POINTER(ctypes.c_int64),
        ctypes.c_size_t,
    ]
    lib.axon_start_nrt_profile.restype = ctypes.c_int64
    lib.axon_stop_nrt_profile.argtypes = [ctypes.c_char_p]
    lib.axon_stop_nrt_profile.restype = ctypes.c_int64

    @contextlib.contextmanager
    def _hook(output_dir, device_ids):
        import jax

        jax.devices()
        if device_ids:
            ids = (ctypes.c_int64 * len(device_ids))(*device_ids)
            rc = lib.axon_start_nrt_profile(ids, len(device_ids))
        else:
            rc = lib.axon_start_nrt_profile(None, 0)
        if rc != 0:
            raise RuntimeError(f"axon_start_nrt_profile rc={rc}")
        try:
            yield
        finally:
            n = lib.axon_stop_nrt_profile(str(output_dir).encode())
            print(f"ntff profile: {n} file(s) -> {output_dir}", file=sys.stderr)

    return _hook


if "antenv.axon_hooks" not in sys.modules:
    _m = types.ModuleType("antenv.axon_hooks")
    _the_hook = _make_ntff_hook()
    _m.get_axon_ntff_profile_hook = lambda: _the_hook
    _m.set_axon_ntff_profile_hook = lambda h: None
    sys.modules["antenv.axon_hooks"] = _m

bass_utils.upload_artifacts = lambda tmpdir: f"file://{tmpdir}"

_WSPLIT_N = [0]


def _split_multi_waits(nc: bass.Bass) -> int:
    """This walrus build accepts at most one sem-wait per instruction (and
    none on Drain). Hoist extra waits onto same-engine NOPs placed just
    before the instruction - engine stalls at the NOP instead, semantics
    unchanged."""
    n_split = 0
    for f in nc.m.functions:
        for blk in f.blocks:
            insts = blk.instructions
            new_list = []
            for inst in insts:
                si = inst.sync_info
                waits = list(si.on_wait) if (si is not None and si.on_wait) else []
                keep = 0 if inst.opcode == "Drain" else 1
                if len(waits) > keep:
                    moved = waits[: len(waits) - keep]
                    for w in moved:
                        _WSPLIT_N[0] += 1
                        nop = mybir.InstNoOp(
                            name=f"wsplit-{_WSPLIT_N[0]}", ins=[], outs=[]
                        )
                        nop.engine = inst.engine
                        nop.sync_info = mybir.SyncInfo(on_wait=[w], on_update=[])
                        new_list.append(nop)
                        n_split += 1
                    si.on_wait = waits[len(waits) - keep :]
                new_list.append(inst)
            if len(new_list) != len(insts):
                insts[:] = new_list
    return n_split


# --------------------------------------------------------------------------
# Kernel build
# --------------------------------------------------------------------------
def build_nc() -> bass.Bass:
    nc = bass.Bass()
    coords = nc.declare_dram_parameter("coords", [P2, 4], F32, isOutput=False)
    out = nc.declare_dram_parameter("out", [2, H, W], F32, isOutput=True)

    with TileContext(nc) as tc:
        with (
            tc.tile_pool(name="tables", bufs=1) as tpool,
            tc.tile_pool(name="acc", bufs=1) as apool,
            tc.tile_pool(name="psum", bufs=2, space="PSUM") as ppool,
        ):
            # ---- per-point 1-D factor tables ----
            ct = tpool.tile([P2, 4], F32)
            nc.sync.dma_start(ct[:], coords[:])
            px = ct[:, 0:1]
            py = ct[:, 1:2]
            r = ct[:, 3:4]

            vx = tpool.tile([P2, 1], F32)
            vy = tpool.tile([P2, 1], F32)
            v = tpool.tile([P2, 1], F32)
            nc.vector.tensor_scalar(vx[:], px, 0.0, None, OP.is_ge)
            nc.vector.tensor_scalar(vy[:], py, 0.0, None, OP.is_ge)
            nc.vector.tensor_tensor(v[:], vx[:], vy[:], OP.mult)

            r2 = tpool.tile([P2, 1], F32)
            inv = tpool.tile([P2, 1], F32)
            nc.vector.tensor_tensor(r2[:], r, r, OP.mult)
            nc.vector.reciprocal(inv[:], r2[:])

            # s = valid ? -1/r^2 : ~-1e30   (kills invalid points: exp -> 0)
            t1 = tpool.tile([P2, 1], F32)
            vm1 = tpool.tile([P2, 1], F32)
            s = tpool.tile([P2, 1], F32)
            nc.vector.tensor_tensor(t1[:], inv[:], v[:], OP.mult)
            nc.vector.tensor_scalar(vm1[:], v[:], -1.0, None, OP.add)
            nc.vector.scalar_tensor_tensor(
                s[:], vm1[:], 1.0e30, t1[:], OP.mult, OP.subtract
            )

            idx = tpool.tile([P2, W], I32)
            idxf = tpool.tile([P2, W], F32)
            nc.gpsimd.iota(idx[:], pattern=[[1, W]], base=0, channel_multiplier=0)
            nc.vector.tensor_copy(idxf[:], idx[:])

            tabA = tpool.tile([P2, H], F32)
            tabB = tpool.tile([P2, W], F32)
            dA = tpool.tile([P2, H], F32)
            dB = tpool.tile([P2, W], F32)
            # d = i - px ; arg = (d * s) * d ; tab = exp(arg)
            nc.vector.tensor_scalar(dA[:], idxf[:], px, None, OP.subtract)
            nc.vector.scalar_tensor_tensor(dA[:], dA[:], s[:], dA[:], OP.mult, OP.mult)
            nc.scalar.activation(tabA[:], dA[:], AF.Exp)
            nc.vector.tensor_scalar(dB[:], idxf[:], py, None, OP.subtract)
            nc.vector.scalar_tensor_tensor(dB[:], dB[:], s[:], dB[:], OP.mult, OP.mult)
            nc.scalar.activation(tabB[:], dB[:], AF.Exp)

            # PE operands must sit at base partition 0/32/64: repack each
            # channel's 24 tables into one flat row (ch0 -> partition 0,
            # ch1 -> partition 32).
            Aflat = tpool.tile([64, HALF * H], F32)
            Bflat = tpool.tile([64, HALF * W], F32)
            for c in range(2):
                bp = c * 32
                nc.sync.dma_start(
                    Aflat[bp : bp + 1, :].rearrange("o (p j) -> o p j", j=H),
                    tabA[c * HALF : (c + 1) * HALF, :],
                )
                nc.sync.dma_start(
                    Bflat[bp : bp + 1, :].rearrange("o (p j) -> o p j", j=W),
                    tabB[c * HALF : (c + 1) * HALF, :],
                )

            Ar = Aflat[:].bitcast(F32R)
            Br = Bflat[:].bitcast(F32R)

            # ---- accumulate outer products per channel ----
            for c in range(2):
                bp = c * 32
                acc = apool.tile([128, 4, W], F32, tag="acc")
                for p in range(HALF):
                    ps = ppool.tile([128, 4, W], F32, tag="ps")
                    for t in range(4):
                        nc.tensor.matmul(
                            ps[:, t, :],
                            Ar[bp : bp + 1, p * H + t * 128 : p * H + (t + 1) * 128],
                            Br[bp : bp + 1, p * W : (p + 1) * W],
                            start=True,
                            stop=True,
                        )
                    if p == 0:
                        nc.vector.tensor_copy(acc[:], ps[:])
                    else:
                        nc.vector.tensor_tensor(acc[:], ps[:], acc[:], OP.max)
                # rows r = t*128 + partition  ->  out[c, r, :]
                nc.sync.dma_start(
                    out[c].rearrange("(t p) j -> p t j", p=128), acc[:]
                )
    _split_multi_waits(nc)
    return nc


_NC_CACHE: bass.Bass | None = None


def _get_nc() -> bass.Bass:
    global _NC_CACHE
    if _NC_CACHE is None:
        _NC_CACHE = build_nc()
    return _NC_CACHE


def run(coords_full: np.ndarray, trace: bool = False):
    """coords_full: [8, 48, 4] float32. Returns ([8,2,H,W] float32, results)."""
    nc = _get_nc()
    in_maps = [
        {"coords": np.ascontiguousarray(coords_full[b], dtype=np.float32)}
        for b in range(B)
    ]
    res = run_bass_kernel_spmd(nc, in_maps, list(range(N_CORES)), trace=trace)
    outs = np.stack([res.results[b]["out"] for b in range(B)], axis=0)
    return outs.astype(np.float32), res


def kernel(x: np.ndarray, coords: np.ndarray) -> np.ndarray:
    out, _ = run(np.asarray(coords), trace=False)
    return out
